# revision 1
# baseline (speedup 1.0000x reference)
"""Trainium2 Bass kernel for a DARTS RNN cell (T=256 steps, B=256, nhid=256).

Strategy
--------
Data-parallel over batch: 8 NeuronCores x 32 batch elements each; the tiny
weights (W0 [512,512], Ws [8,256,512]) are replicated. The T=256 recurrence is
sequential, computed fully on-chip.

Per-core layout is feature-major ("transposed"): every state tensor s^T lives
in one SBUF tile [128 partitions, 64] = (feature f%128 on partitions,
32*(f//128) + b on the free dim). Matmuls then run with the weight chunk
[128,128] (bf16, FWL fast weight load) stationary and the state chunk [128,32]
moving, producing feature-major PSUM directly -- no on-chip transposes at all.
The host pre-transposes/casts x, pre-chunks the weights into the exact SBUF
layout, and re-transposes the feature-major output.

Numerics: matmul operands bf16, accumulation + state updates + activations
fp32. (Measured vs the fp32 reference: rel l2 err ~6e-4, flat over t.)
The identity-activation step folds (W - I) into the weights host-side so the
"h - s" subtraction comes straight out of PSUM.
"""

import numpy as np
import ml_dtypes
from contextlib import ExitStack

import concourse.bass as bass
import concourse.tile as tile
from concourse import bacc, mybir
from concourse.bass_utils import run_bass_kernel_spmd

BF16 = ml_dtypes.bfloat16

GENOTYPE_RNN = [("sigmoid", 0), ("relu", 1), ("relu", 1), ("identity", 1),
                ("tanh", 2), ("sigmoid", 5), ("tanh", 3), ("relu", 5)]
T, B, NINP, NHID = 256, 256, 256, 256
N_CORES = 8
BL = B // N_CORES          # 32 batch elements per core
N_CHUNKS = 16 + 8 * 8      # W0 (4k x 4m) + 8 genotype (2k x 4m)
PREDS_USED = {0, 1, 2, 3, 5}   # states needed (bf16) as matmul moving operands

_ACT_FN = {"sigmoid": "Sigmoid", "tanh": "Tanh"}


def _chunk_index(step, k, m):
    """Column-chunk index of weight block (step, k, m) in the packed w_sb."""
    base = 0 if step == 0 else 16 + 8 * (step - 1)
    return base + k * 4 + m


def _pack_weights(W0, Ws):
    """Pack W0 / Ws (identity-folded for the identity step) into the SBUF
    layout [128, N_CHUNKS*128] bf16, chunk j at columns [128j, 128j+128)."""
    Wsf = np.array(Ws, dtype=np.float32, copy=True)
    for i, (name, _pred) in enumerate(GENOTYPE_RNN):
        if name == "identity":
            Wsf[i][:, NHID:] -= np.eye(NHID, dtype=np.float32)
    w = np.zeros((128, N_CHUNKS * 128), dtype=BF16)
    for k in range(4):
        for m in range(4):
            j = _chunk_index(0, k, m)
            w[:, 128 * j:128 * (j + 1)] = W0[128 * k:128 * (k + 1),
                                             128 * m:128 * (m + 1)].astype(BF16)
    for i in range(8):
        for k in range(2):
            for m in range(4):
                j = _chunk_index(i + 1, k, m)
                w[:, 128 * j:128 * (j + 1)] = Wsf[i][128 * k:128 * (k + 1),
                                                     128 * m:128 * (m + 1)].astype(BF16)
    return w


def _to_fm(a):
    """[T?, b, f] batch-major -> feature-major [T?, 128, 2, b] device layout."""
    a = np.asarray(a, dtype=np.float32)
    if a.ndim == 2:                      # [b, f]
        b, f = a.shape
        return a.T.reshape(2, 128, b).transpose(1, 0, 2)
    t, b, f = a.shape                    # [T, b, f]
    return a.transpose(0, 2, 1).reshape(t, 2, 128, b).transpose(0, 2, 1, 3)


def _from_fm(a):
    """[T, 128, 2, b] feature-major device layout -> [T, b, f]."""
    t = a.shape[0]
    return a.transpose(0, 2, 1, 3).reshape(t, NHID, -1).transpose(0, 2, 1)


def _build_program(n_t=T):
    """Build + compile the per-core Bass/Tile program (identical on all cores)."""
    f32 = mybir.dt.float32
    bf16 = mybir.dt.bfloat16
    AF = mybir.ActivationFunctionType
    ALU = mybir.AluOpType

    nc = bacc.Bacc("TRN2", target_bir_lowering=False, debug=False,
                   enable_asserts=False, enable_partition_id=False)

    x_d = nc.dram_tensor("x_fm", [n_t, 128, 2, BL], bf16, kind="ExternalInput").ap()
    w_d = nc.dram_tensor("w_sb", [128, N_CHUNKS * 128], bf16, kind="ExternalInput").ap()
    h0f_d = nc.dram_tensor("h0_f32", [128, 2, BL], f32, kind="ExternalInput").ap()
    h0b_d = nc.dram_tensor("h0_bf", [128, 2, BL], bf16, kind="ExternalInput").ap()
    out_d = nc.dram_tensor("out_fm", [n_t, 128, 2, BL], f32, kind="ExternalOutput").ap()

    FD = 2 * BL    # 64: free dim of one state tile

    with tile.TileContext(nc) as tc, ExitStack() as ctx:
        wpool = ctx.enter_context(tc.tile_pool(name="w", bufs=1))
        xpool = ctx.enter_context(tc.tile_pool(name="x", bufs=8))
        spool = ctx.enter_context(tc.tile_pool(name="s", bufs=3))
        bfpool = ctx.enter_context(tc.tile_pool(name="sbf", bufs=3))
        tpool = ctx.enter_context(tc.tile_pool(name="tmp", bufs=4))
        opool = ctx.enter_context(tc.tile_pool(name="out", bufs=4))
        pspool = ctx.enter_context(tc.tile_pool(name="ps", bufs=6, space="PSUM"))

        w = wpool.tile([128, N_CHUNKS * 128], bf16, tag="w")
        nc.sync.dma_start(w, w_d)

        def wap(step, k, m):
            j = _chunk_index(step, k, m)
            return w[:, 128 * j:128 * (j + 1)]

        hprev_f = opool.tile([128, FD], f32, tag="hf")
        hprev_b = bfpool.tile([128, FD], bf16, tag="hb")
        nc.sync.dma_start(hprev_f.rearrange("p (c b) -> p c b", c=2), h0f_d)
        nc.sync.dma_start(hprev_b.rearrange("p (c b) -> p c b", c=2), h0b_d)

        for t in range(n_t):
            xt = xpool.tile([128, FD], bf16, tag="x")
            nc.sync.dma_start(xt.rearrange("p (c b) -> p c b", c=2), x_d[t])

            states_f = [None] * 9
            states_b = {}

            # ---- init step: ch0 = [x, h_prev] @ W0 (feature-major) ----
            ps = pspool.tile([128, 4 * BL], f32, tag="ps")
            first = True
            for m in range(4):
                for k in range(4):
                    src = xt if k < 2 else hprev_b
                    rhs = src[:, BL * (k % 2):BL * (k % 2 + 1)]
                    nc.tensor.matmul(ps[:, BL * m:BL * (m + 1)], lhsT=wap(0, k, m),
                                     rhs=rhs, start=first, stop=(m == 3 and k == 3))
                    first = False
            c = tpool.tile([128, FD], f32, tag="c")
            h = tpool.tile([128, FD], f32, tag="h")
            nc.scalar.activation(c, ps[:, 0:FD], AF.Sigmoid)
            nc.scalar.activation(h, ps[:, FD:2 * FD], AF.Tanh)
            d = tpool.tile([128, FD], f32, tag="d")
            nc.vector.tensor_sub(d, h, hprev_f)
            e = tpool.tile([128, FD], f32, tag="e")
            nc.vector.tensor_mul(e, c, d)
            s = spool.tile([128, FD], f32, tag="s0")
            nc.vector.tensor_add(s, hprev_f, e)
            states_f[0] = s
            sb = bfpool.tile([128, FD], bf16, tag="s0b")
            nc.gpsimd.tensor_copy(sb, s)
            states_b[0] = sb

            # ---- genotype steps ----
            for i, (name, pred) in enumerate(GENOTYPE_RNN):
                ps = pspool.tile([128, 4 * BL], f32, tag="ps")
                first = True
                for m in range(4):
                    for k in range(2):
                        nc.tensor.matmul(ps[:, BL * m:BL * (m + 1)],
                                         lhsT=wap(i + 1, k, m),
                                         rhs=states_b[pred][:, BL * k:BL * (k + 1)],
                                         start=first, stop=(m == 3 and k == 1))
                        first = False
                spf = states_f[pred]
                c = tpool.tile([128, FD], f32, tag="c")
                nc.scalar.activation(c, ps[:, 0:FD], AF.Sigmoid)
                if name in _ACT_FN:
                    h = tpool.tile([128, FD], f32, tag="h")
                    nc.scalar.activation(h, ps[:, FD:2 * FD], getattr(AF, _ACT_FN[name]))
                    d = tpool.tile([128, FD], f32, tag="d")
                    nc.vector.tensor_sub(d, h, spf)
                elif name == "relu":
                    d = tpool.tile([128, FD], f32, tag="d")
                    nc.vector.scalar_tensor_tensor(d, ps[:, FD:2 * FD], 0.0, spf,
                                                   op0=ALU.max, op1=ALU.subtract)
                else:  # identity: (W - I) folded host-side, psum already holds h - s
                    d = ps[:, FD:2 * FD]
                e = tpool.tile([128, FD], f32, tag="e")
                nc.vector.tensor_mul(e, c, d)
                s = spool.tile([128, FD], f32, tag=f"s{i + 1}")
                nc.vector.tensor_add(s, spf, e)
                states_f[i + 1] = s
                if (i + 1) in PREDS_USED:
                    sb = bfpool.tile([128, FD], bf16, tag=f"s{i + 1}b")
                    nc.gpsimd.tensor_copy(sb, s)
                    states_b[i + 1] = sb

            # ---- h_new = mean(s1..s8) ----
            a1 = tpool.tile([128, FD], f32, tag="a1")
            nc.gpsimd.tensor_add(a1, states_f[1], states_f[2])
            a2 = tpool.tile([128, FD], f32, tag="a2")
            nc.gpsimd.tensor_add(a2, states_f[3], states_f[4])
            a3 = tpool.tile([128, FD], f32, tag="a3")
            nc.gpsimd.tensor_add(a3, states_f[5], states_f[6])
            a4 = tpool.tile([128, FD], f32, tag="a4")
            nc.gpsimd.tensor_add(a4, states_f[7], states_f[8])
            b1 = tpool.tile([128, FD], f32, tag="b1")
            nc.vector.tensor_add(b1, a1, a2)
            b2 = tpool.tile([128, FD], f32, tag="b2")
            nc.vector.tensor_add(b2, a3, a4)
            hsum = tpool.tile([128, FD], f32, tag="hs")
            nc.vector.tensor_add(hsum, b1, b2)

            hprev_f = opool.tile([128, FD], f32, tag="hf")
            nc.vector.tensor_scalar_mul(hprev_f, hsum, 0.125)
            hprev_b = bfpool.tile([128, FD], bf16, tag="hb")
            nc.gpsimd.tensor_copy(hprev_b, hprev_f)

            nc.sync.dma_start(out_d[t], hprev_f.rearrange("p (c b) -> p c b", c=2))

    nc.compile()
    return nc


_PROGRAM_CACHE = {}


def get_program(n_t=T):
    if n_t not in _PROGRAM_CACHE:
        _PROGRAM_CACHE[n_t] = _build_program(n_t)
    return _PROGRAM_CACHE[n_t]


def make_in_maps(inputs_x, hidden, W0, Ws):
    """Host-side prep: shard batch, pack weights, transpose to device layout."""
    w_sb = _pack_weights(np.asarray(W0, np.float32), np.asarray(Ws, np.float32))
    x = np.asarray(inputs_x, np.float32)
    h0 = np.asarray(hidden, np.float32)[0]
    in_maps = []
    for core in range(N_CORES):
        sl = slice(core * BL, (core + 1) * BL)
        h0_fm = _to_fm(h0[sl])
        in_maps.append({
            "x_fm": _to_fm(x[:, sl, :]).astype(BF16),
            "w_sb": w_sb,
            "h0_f32": h0_fm,
            "h0_bf": h0_fm.astype(BF16),
        })
    return in_maps


def run(inputs, hidden, W0, Ws, n_t=T, trace=False, **spmd_kwargs):
    nc = get_program(n_t)
    in_maps = make_in_maps(inputs, hidden, W0, Ws)
    res = run_bass_kernel_spmd(nc, in_maps, core_ids=list(range(N_CORES)),
                               trace=trace, **spmd_kwargs)
    hid = np.concatenate([_from_fm(r["out_fm"]) for r in res.results], axis=1)
    return hid.astype(np.float32), res


def kernel(inputs, hidden, W0, Ws, rnn_mask=None, **_ignored):
    hiddens, _res = run(inputs, hidden, W0, Ws)
    return hiddens, hiddens[-1][None]
